# revision 14
# baseline (speedup 1.0000x reference)
"""Trainium2 Bass kernel for single-head causal attention with dropout.

reference:
    q,k,v = x@Wq, x@Wk, x@Wv          [B,T,H]
    wei = softmax(mask(q@k^T * H**-0.5))   (causal)
    wei = wei * (drop_u >= 0.2)/0.8
    out = wei @ v                      [B,T,H]

B=16, T=2048, D=1024, H=64. 8 NeuronCores, data-parallel over batch
(2 batches/core). Matmuls run in float32r (TF32).

Layout strategy: x and drop_u are shipped pre-transposed ([B,D,T] /
[B,T_s,T_q]) so everything on-chip runs in the "scores-transposed"
orientation S^T[s, q]: chunks of [128 keys x 512 queries] stream
through S^T-matmul -> +causal-mask-matmul -> ScalarE exp -> fused
VectorE dropout, feeding two PSUM accumulations per query group:
out^T += v_chunk.T @ P'^T  and  denom += ones.T @ E^T. No transposes
of the attention matrix are ever needed.
"""

import numpy as np
from contextlib import ExitStack

import concourse.bass as bass
import concourse.tile as tile
from concourse import mybir
from concourse.bass_utils import run_bass_kernel_spmd
from concourse.masks import make_identity

F32 = mybir.dt.float32
F32R = mybir.dt.float32r
BF16 = mybir.dt.bfloat16

B, T, D, H = 16, 2048, 1024, 64
N_CORES = 8
BPC = B // N_CORES          # batches per core
P_DROP = 0.2
NB = T // 128               # 16 key chunks per batch
NG = T // 512               # 4 query groups per batch
GROUP = 4                   # key chunks per query group


# walrus here allows only ONE sync-wait command per instruction; Tile can
# attach several (e.g. its exit drain). Move extras onto same-engine NOPs.
def _split_excess_waits(nc):
    n = 0
    for f in nc.m.functions:
        for bb in f.blocks:
            new_insts = []
            changed = False
            for inst in bb.instructions:
                si = inst.sync_info
                if si is not None and si.on_wait and len(si.on_wait) > 1:
                    waits = list(si.on_wait)
                    extra, keep = waits[:-1], waits[-1:]
                    for i, w in enumerate(extra):
                        new_insts.append(mybir.InstNoOp(
                            name=f"{inst.name}-ws-{i}",
                            engine=inst.engine, ins=[], outs=[],
                            sync_info=mybir.SyncInfo(on_wait=[w], on_update=[]),
                            text_hint="waitsplit", bass_nofuse=True))
                        n += 1
                    si.on_wait = keep
                    changed = True
                new_insts.append(inst)
            if changed:
                bb.instructions[:] = new_insts
    return n


def _build(ctx: ExitStack, tc: "tile.TileContext", xt, wqk, wv, ut, out):
    nc = tc.nc
    AF = mybir.ActivationFunctionType
    OP = mybir.AluOpType

    cpool = ctx.enter_context(tc.tile_pool(name="const", bufs=1))
    xpool = ctx.enter_context(tc.tile_pool(name="xt", bufs=2))
    qkvpool = ctx.enter_context(tc.tile_pool(name="qkv", bufs=2))
    vtpool = ctx.enter_context(tc.tile_pool(name="vt", bufs=1))
    epool = ctx.enter_context(tc.tile_pool(name="e", bufs=4))
    pppool = ctx.enter_context(tc.tile_pool(name="pp", bufs=4))
    upool = ctx.enter_context(tc.tile_pool(name="u", bufs=6))
    otsbpool = ctx.enter_context(tc.tile_pool(name="otsb", bufs=2))
    onsbpool = ctx.enter_context(tc.tile_pool(name="onsb", bufs=2))
    outpool = ctx.enter_context(tc.tile_pool(name="outsb", bufs=3))
    rdpool = ctx.enter_context(tc.tile_pool(name="rd", bufs=2))

    projps = ctx.enter_context(tc.tile_pool(name="projps", bufs=2, space="PSUM"))
    stps = ctx.enter_context(tc.tile_pool(name="stps", bufs=3, space="PSUM"))
    dps_pool = ctx.enter_context(tc.tile_pool(name="dps", bufs=1, space="PSUM"))
    otps = ctx.enter_context(tc.tile_pool(name="otps", bufs=1, space="PSUM"))
    stageps = ctx.enter_context(tc.tile_pool(name="stage", bufs=1, space="PSUM"))

    # ---- constants -------------------------------------------------------
    ident_f = cpool.tile([128, 128], F32)
    make_identity(nc, ident_f[:])
    ident_r = cpool.tile([128, 128], F32R)
    nc.vector.tensor_copy(ident_r[:], ident_f[:])
    identb = cpool.tile([128, 128], BF16)
    make_identity(nc, identb[:])

    # transposed block causal mask: keep (0) where s <= q, -1e10 where s > q
    cmaskT = cpool.tile([128, 128], BF16)
    nc.gpsimd.memset(cmaskT[:], 0.0)
    nc.gpsimd.affine_select(
        out=cmaskT[:], in_=cmaskT[:], compare_op=OP.is_ge, fill=-1e10,
        base=0, pattern=[[1, 128]], channel_multiplier=-1)

    ones_f = cpool.tile([128, 1], F32)
    nc.gpsimd.memset(ones_f[:], 1.0)
    ones_r = cpool.tile([128, 1], F32R)
    nc.vector.tensor_copy(ones_r[:], ones_f[:])
    c125_f = cpool.tile([1, 64], F32)
    nc.gpsimd.memset(c125_f[:], 1.0 / (1.0 - P_DROP))
    c125 = cpool.tile([1, 64], F32R)
    nc.vector.tensor_copy(c125[:], c125_f[:])

    wqk_sb = cpool.tile([128, 8 * 128], F32R)
    nc.sync.dma_start(
        wqk_sb[:].rearrange("p (c h) -> p c h", c=8),
        wqk.rearrange("(c p) h -> p c h", p=128))
    wv_sb = cpool.tile([128, 8 * H], F32R)
    nc.sync.dma_start(
        wv_sb[:].rearrange("p (c h) -> p c h", c=8),
        wv.rearrange("(c p) h -> p c h", p=128))

    for b in range(BPC):
        # ---- phase A: projections ---------------------------------------
        # qkT[0:64,:] = q^T, qkT[64:128,:] = k^T ; v natural [s, H] tiles
        qkT = qkvpool.tile([128, T], F32R, tag="qkT")
        kT0 = qkvpool.tile([64, T], F32R, tag="kT0")
        vT = vtpool.tile([64, T], F32R, tag="vT")
        v_sb = qkvpool.tile([128, NB * H], F32R, tag="v")

        for quarter in range(4):
            col = 512 * quarter
            xts = []
            for c in range(8):
                xt_c = xpool.tile([128, 512], F32R, tag=f"xt{c}")
                nc.gpsimd.dma_start(
                    xt_c[:], xt[b, 128 * c:128 * (c + 1), col:col + 512])
                xts.append(xt_c)
            ps = projps.tile([128, 512], F32, tag="projps")
            for c in range(8):
                nc.tensor.matmul(
                    ps[:], wqk_sb[:, 128 * c:128 * (c + 1)], xts[c][:],
                    start=(c == 0), stop=(c == 7))
            nc.scalar.copy(qkT[:, col:col + 512], ps[:])
            # matmul needs lhsT/rhs at the same base partition: move k^T
            # (psum rows 64..127) down to partitions 0..63 via DMA
            nc.sync.dma_start(kT0[:, col:col + 512], qkT[64:128, col:col + 512])
            ps2 = projps.tile([64, 512], F32, tag="projps")
            for c in range(8):
                nc.tensor.matmul(
                    ps2[:], wv_sb[:, H * c:H * (c + 1)], xts[c][:],
                    start=(c == 0), stop=(c == 7))
            nc.scalar.copy(vT[:, col:col + 512], ps2[:])
        qT = qkT
        kT = kT0

        # v: [64,T] -> natural [s, H] tiles, 8 transposes per PSUM bank
        for m in range(2):
            stage = stageps.tile([128, 512], F32R, tag="stage")
            for tloc in range(8):
                t = 8 * m + tloc
                nc.tensor.transpose(
                    stage[:, H * tloc:H * (tloc + 1)],
                    vT[:, 128 * t:128 * (t + 1)], ident_r[:64, :64])
            nc.vector.tensor_copy(
                v_sb[:, H * 8 * m:H * 8 * (m + 1)], stage[:])

        # ---- phase B: attention, per query group of 512 ------------------
        for g in range(NG):
            qcol = 512 * g
            nchunks = GROUP * (g + 1)
            dps = dps_pool.tile([1, 512], F32, tag="dps")
            ot = otps.tile([64, 512], F32, tag="ot")

            for t in range(nchunks):
                qo = 128 * max(0, t - GROUP * g)   # causal offset in group
                n = 512 - qo
                u_t = upool.tile([128, 512], F32, tag="u")
                nc.sync.dma_start(
                    u_t[:, qo:512],
                    ut[b, 128 * t:128 * (t + 1), qcol + qo:qcol + 512])

                sps = stps.tile([128, 512], F32, tag="S")
                nc.tensor.matmul(
                    sps[:, qo:512], kT[:, 128 * t:128 * (t + 1)],
                    qT[0:64, qcol + qo:qcol + 512], start=True,
                    stop=(t < GROUP * g))
                if t >= GROUP * g:   # block-diagonal: add triangular mask
                    nc.tensor.matmul(
                        sps[:, qo:qo + 128], identb[:], cmaskT[:],
                        start=False, stop=True, skip_group_check=True)

                E = epool.tile([128, 512], F32R, tag="E")
                nc.scalar.activation(
                    E[:, qo:512], sps[:, qo:512], AF.Exp,
                    scale=float(H) ** -0.5)
                # denom += ones^T @ E^T   (pre-dropout row sums, on PE)
                nc.tensor.matmul(
                    dps[0:1, qo:512], ones_r[:], E[:, qo:512],
                    start=(t == 0), stop=(t == nchunks - 1))
                # dropout: P'^T = (u^T >= p) * E^T
                Pp = pppool.tile([128, 512], F32R, tag="Pp")
                nc.vector.scalar_tensor_tensor(
                    Pp[:, qo:512], u_t[:, qo:512], P_DROP,
                    E[:, qo:512].bitcast(F32),
                    op0=OP.is_ge, op1=OP.mult)
                # out^T += v_chunk^T @ P'^T
                nc.tensor.matmul(
                    ot[:, qo:512], v_sb[:, H * t:H * (t + 1)], Pp[:, qo:512],
                    start=(t == 0), stop=(t == nchunks - 1))

            # ---- group epilogue -----------------------------------------
            rd = rdpool.tile([1, 512], F32R, tag="rd")
            with nc.allow_low_precision(reason="tf32 softmax denominator"):
                nc.vector.reciprocal(rd[:], dps[:])
            rdbc = stageps.tile([64, 512], F32, tag="stage")
            nc.tensor.matmul(rdbc[:], c125[:], rd[:], start=True, stop=True)
            ot_sb = otsbpool.tile([64, 512], F32, tag="otsb")
            nc.scalar.copy(ot_sb[:], ot[:])
            on_sb = onsbpool.tile([64, 512], F32, tag="onsb")
            nc.vector.tensor_mul(on_sb[:], ot_sb[:], rdbc[:])
            # transpose back to [q, H] and store
            onat = stageps.tile([128, 256], F32, tag="stage")
            for cc in range(GROUP):
                nc.tensor.transpose(
                    onat[:, 64 * cc:64 * (cc + 1)],
                    on_sb[:, 128 * cc:128 * (cc + 1)], ident_f[:64, :64])
            osb = outpool.tile([128, 256], F32, tag="osb")
            nc.vector.tensor_copy(osb[:], onat[:])
            nc.sync.dma_start(
                out[b].rearrange("(c p) h -> p c h", p=128)
                   [:, GROUP * g:GROUP * (g + 1), :],
                osb[:].rearrange("p (c h) -> p c h", c=GROUP))


_CACHE = {}


def _get_nc():
    if "nc" not in _CACHE:
        nc = bass.Bass("TRN2", target_bir_lowering=False)
        xt = nc.dram_tensor("xt", [BPC, D, T], F32R, kind="ExternalInput")
        wqk = nc.dram_tensor("wqk", [D, 128], F32R, kind="ExternalInput")
        wv = nc.dram_tensor("wv", [D, H], F32R, kind="ExternalInput")
        ut = nc.dram_tensor("ut", [BPC, T, T], F32, kind="ExternalInput")
        out = nc.dram_tensor("out", [BPC, T, H], F32, kind="ExternalOutput")
        with tile.TileContext(nc) as tc:
            with ExitStack() as ctx:
                _build(ctx, tc, xt.ap(), wqk.ap(), wv.ap(), ut.ap(), out.ap())
        _split_excess_waits(nc)
        _CACHE["nc"] = nc
    return _CACHE["nc"]


def kernel(x, Wq, Wk, Wv, drop_u, _trace=False):
    x = np.asarray(x, dtype=np.float32)
    Wq = np.asarray(Wq, dtype=np.float32)
    Wk = np.asarray(Wk, dtype=np.float32)
    Wv = np.asarray(Wv, dtype=np.float32)
    drop_u = np.asarray(drop_u, dtype=np.float32)

    nc = _get_nc()
    xt = np.ascontiguousarray(x.transpose(0, 2, 1))        # [B, D, T]
    ut = np.ascontiguousarray(drop_u.transpose(0, 2, 1))   # [B, T_s, T_q]
    wqk = np.ascontiguousarray(np.concatenate([Wq, Wk], axis=1))  # [D, 128]
    in_maps = []
    for c in range(N_CORES):
        lo = BPC * c
        in_maps.append({
            "xt": xt[lo:lo + BPC],
            "wqk": wqk, "wv": Wv,
            "ut": ut[lo:lo + BPC],
        })
    res = run_bass_kernel_spmd(
        nc, in_maps, core_ids=list(range(N_CORES)), trace=_trace)
    out = np.concatenate([res.results[c]["out"] for c in range(N_CORES)], axis=0)
    if _trace:
        kernel.last_exec_time_ns = res.exec_time_ns
        kernel.last_results = res
    return out


# revision 17
# speedup vs baseline: 1.1275x; 1.1275x over previous
"""Trainium2 Bass kernel for single-head causal attention with dropout.

reference:
    q,k,v = x@Wq, x@Wk, x@Wv          [B,T,H]
    wei = softmax(mask(q@k^T * H**-0.5))   (causal)
    wei = wei * (drop_u >= 0.2)/0.8
    out = wei @ v                      [B,T,H]

B=16, T=2048, D=1024, H=64. 8 NeuronCores, data-parallel over batch
(2 batches/core). Matmuls run in float32r (TF32).

Layout strategy: x and drop_u are shipped pre-transposed ([B,D,T] /
[B,T_s,T_q]) so everything on-chip runs in the "scores-transposed"
orientation S^T[s, q]: chunks of [128 keys x 512 queries] stream
through S^T-matmul -> +causal-mask-matmul -> ScalarE exp -> fused
VectorE dropout, feeding two PSUM accumulations per query group:
out^T += v_chunk.T @ P'^T  and  denom += ones.T @ E^T. No transposes
of the attention matrix are ever needed.
"""

import numpy as np
from contextlib import ExitStack

import concourse.bass as bass
import concourse.tile as tile
from concourse import mybir
from concourse.bass_utils import run_bass_kernel_spmd
from concourse.masks import make_identity

F32 = mybir.dt.float32
F32R = mybir.dt.float32r
BF16 = mybir.dt.bfloat16

B, T, D, H = 16, 2048, 1024, 64
N_CORES = 8
BPC = B // N_CORES          # batches per core
P_DROP = 0.2
NB = T // 128               # 16 key chunks per batch
NG = T // 512               # 4 query groups per batch
GROUP = 4                   # key chunks per query group


# walrus here allows only ONE sync-wait command per instruction; Tile can
# attach several (e.g. its exit drain). Move extras onto same-engine NOPs.
def _split_excess_waits(nc):
    n = 0
    for f in nc.m.functions:
        for bb in f.blocks:
            new_insts = []
            changed = False
            for inst in bb.instructions:
                si = inst.sync_info
                if si is not None and si.on_wait and len(si.on_wait) > 1:
                    waits = list(si.on_wait)
                    extra, keep = waits[:-1], waits[-1:]
                    for i, w in enumerate(extra):
                        new_insts.append(mybir.InstNoOp(
                            name=f"{inst.name}-ws-{i}",
                            engine=inst.engine, ins=[], outs=[],
                            sync_info=mybir.SyncInfo(on_wait=[w], on_update=[]),
                            text_hint="waitsplit", bass_nofuse=True))
                        n += 1
                    si.on_wait = keep
                    changed = True
                new_insts.append(inst)
            if changed:
                bb.instructions[:] = new_insts
    return n


def _build(ctx: ExitStack, tc: "tile.TileContext", xt, wqk, wv, ut, out):
    nc = tc.nc
    AF = mybir.ActivationFunctionType
    OP = mybir.AluOpType

    cpool = ctx.enter_context(tc.tile_pool(name="const", bufs=1))
    xpool = ctx.enter_context(tc.tile_pool(name="xt", bufs=2))
    qkvpool = ctx.enter_context(tc.tile_pool(name="qkv", bufs=2))
    vtpool = ctx.enter_context(tc.tile_pool(name="vt", bufs=1))
    epool = ctx.enter_context(tc.tile_pool(name="e", bufs=4))
    pppool = ctx.enter_context(tc.tile_pool(name="pp", bufs=4))
    upool = ctx.enter_context(tc.tile_pool(name="u", bufs=6))
    otsbpool = ctx.enter_context(tc.tile_pool(name="otsb", bufs=2))
    onsbpool = ctx.enter_context(tc.tile_pool(name="onsb", bufs=2))
    outpool = ctx.enter_context(tc.tile_pool(name="outsb", bufs=3))
    rdpool = ctx.enter_context(tc.tile_pool(name="rd", bufs=2))

    projps = ctx.enter_context(tc.tile_pool(name="projps", bufs=2, space="PSUM"))
    stps = ctx.enter_context(tc.tile_pool(name="stps", bufs=3, space="PSUM"))
    dps_pool = ctx.enter_context(tc.tile_pool(name="dps", bufs=1, space="PSUM"))
    otps = ctx.enter_context(tc.tile_pool(name="otps", bufs=1, space="PSUM"))
    stageps = ctx.enter_context(tc.tile_pool(name="stage", bufs=1, space="PSUM"))

    # ---- constants -------------------------------------------------------
    ident_f = cpool.tile([128, 128], F32)
    make_identity(nc, ident_f[:])
    ident_r = cpool.tile([128, 128], F32R)
    nc.vector.tensor_copy(ident_r[:], ident_f[:])
    identb = cpool.tile([128, 128], BF16)
    make_identity(nc, identb[:])

    # transposed block causal mask: keep (0) where s <= q, -1e10 where s > q
    cmaskT = cpool.tile([128, 128], BF16)
    nc.gpsimd.memset(cmaskT[:], 0.0)
    nc.gpsimd.affine_select(
        out=cmaskT[:], in_=cmaskT[:], compare_op=OP.is_ge, fill=-1e10,
        base=0, pattern=[[1, 128]], channel_multiplier=-1)

    ones_f = cpool.tile([128, 1], F32)
    nc.gpsimd.memset(ones_f[:], 1.0)
    ones_r = cpool.tile([128, 1], F32R)
    nc.vector.tensor_copy(ones_r[:], ones_f[:])
    c125_f = cpool.tile([1, 64], F32)
    nc.gpsimd.memset(c125_f[:], 1.0 / (1.0 - P_DROP))
    c125 = cpool.tile([1, 64], F32R)
    nc.vector.tensor_copy(c125[:], c125_f[:])

    wqk_sb = cpool.tile([128, 8 * 128], F32R)
    nc.sync.dma_start(
        wqk_sb[:].rearrange("p (c h) -> p c h", c=8),
        wqk.rearrange("(c p) h -> p c h", p=128))
    wv_sb = cpool.tile([128, 8 * H], F32R)
    nc.sync.dma_start(
        wv_sb[:].rearrange("p (c h) -> p c h", c=8),
        wv.rearrange("(c p) h -> p c h", p=128))

    for b in range(BPC):
        # ---- phase A: projections ---------------------------------------
        # qkT[0:64,:] = q^T, qkT[64:128,:] = k^T ; v natural [s, H] tiles
        qkT = qkvpool.tile([128, T], F32R, tag="qkT")
        kT0 = qkvpool.tile([64, T], F32R, tag="kT0")
        vT = vtpool.tile([64, T], F32R, tag="vT")
        v_sb = qkvpool.tile([128, NB * H], F32R, tag="v")

        for quarter in range(4):
            col = 512 * quarter
            xts = []
            for c in range(8):
                xt_c = xpool.tile([128, 512], F32R, tag=f"xt{c}")
                nc.gpsimd.dma_start(
                    xt_c[:], xt[b, 128 * c:128 * (c + 1), col:col + 512])
                xts.append(xt_c)
            ps = projps.tile([128, 512], F32, tag="projps")
            for c in range(8):
                nc.tensor.matmul(
                    ps[:], wqk_sb[:, 128 * c:128 * (c + 1)], xts[c][:],
                    start=(c == 0), stop=(c == 7))
            nc.scalar.copy(qkT[:, col:col + 512], ps[:])
            # matmul needs lhsT/rhs at the same base partition: move k^T
            # (psum rows 64..127) down to partitions 0..63 via DMA
            nc.sync.dma_start(kT0[:, col:col + 512], qkT[64:128, col:col + 512])
            ps2 = projps.tile([64, 512], F32, tag="projps")
            for c in range(8):
                nc.tensor.matmul(
                    ps2[:], wv_sb[:, H * c:H * (c + 1)], xts[c][:],
                    start=(c == 0), stop=(c == 7))
            nc.scalar.copy(vT[:, col:col + 512], ps2[:])
        qT = qkT
        kT = kT0

        # v: [64,T] -> natural [s, H] tiles, 8 transposes per PSUM bank
        for m in range(2):
            stage = stageps.tile([128, 512], F32R, tag="stage")
            for tloc in range(8):
                t = 8 * m + tloc
                nc.tensor.transpose(
                    stage[:, H * tloc:H * (tloc + 1)],
                    vT[:, 128 * t:128 * (t + 1)], ident_r[:64, :64])
            nc.vector.tensor_copy(
                v_sb[:, H * 8 * m:H * 8 * (m + 1)], stage[:])

        # ---- phase B: attention, per query group of 512 ------------------
        for g in range(NG):
            qcol = 512 * g
            nchunks = GROUP * (g + 1)
            dps = dps_pool.tile([1, 512], F32, tag="dps")
            ot = otps.tile([64, 512], F32, tag="ot")

            # software-pipelined: consumer stages trail PD chunks so the PE
            # stream (S-matmuls vs denom/out matmuls) never stalls on
            # ScalarE exp / VectorE dropout of the same chunk.
            PD = 2
            uts, es = {}, {}
            for t in range(nchunks + PD):
                if t < nchunks:
                    qo = 128 * max(0, t - GROUP * g)  # causal offset in group
                    u_t = upool.tile([128, 512], F32, tag="u")
                    nc.sync.dma_start(
                        u_t[:, qo:512],
                        ut[b, 128 * t:128 * (t + 1), qcol + qo:qcol + 512])
                    uts[t] = u_t

                    sps = stps.tile([128, 512], F32, tag="S")
                    nc.tensor.matmul(
                        sps[:, qo:512], kT[:, 128 * t:128 * (t + 1)],
                        qT[0:64, qcol + qo:qcol + 512], start=True,
                        stop=(t < GROUP * g))
                    if t >= GROUP * g:  # block-diagonal: add triangular mask
                        nc.tensor.matmul(
                            sps[:, qo:qo + 128], identb[:], cmaskT[:],
                            start=False, stop=True, skip_group_check=True)

                    E = epool.tile([128, 512], F32R, tag="E")
                    nc.scalar.activation(
                        E[:, qo:512], sps[:, qo:512], AF.Exp,
                        scale=float(H) ** -0.5)
                    es[t] = E
                if t >= PD:
                    tt = t - PD
                    qo = 128 * max(0, tt - GROUP * g)
                    E = es.pop(tt)
                    u_t = uts.pop(tt)
                    # denom += ones^T @ E^T   (pre-dropout row sums, on PE)
                    nc.tensor.matmul(
                        dps[0:1, qo:512], ones_r[:], E[:, qo:512],
                        start=(tt == 0), stop=(tt == nchunks - 1))
                    # dropout: P'^T = (u^T >= p) * E^T
                    Pp = pppool.tile([128, 512], F32R, tag="Pp")
                    nc.vector.scalar_tensor_tensor(
                        Pp[:, qo:512], u_t[:, qo:512], P_DROP,
                        E[:, qo:512].bitcast(F32),
                        op0=OP.is_ge, op1=OP.mult)
                    # out^T += v_chunk^T @ P'^T
                    nc.tensor.matmul(
                        ot[:, qo:512], v_sb[:, H * tt:H * (tt + 1)],
                        Pp[:, qo:512],
                        start=(tt == 0), stop=(tt == nchunks - 1))

            # ---- group epilogue -----------------------------------------
            # 1/d as exp(-ln d) on ScalarE (a [1,512] DVE reciprocal would
            # run on a single lane at ~6 cyc/elem)
            ln_d = rdpool.tile([1, 512], F32, tag="rdf")
            nc.scalar.activation(ln_d[:], dps[:], AF.Ln)
            rd = rdpool.tile([1, 512], F32R, tag="rd")
            nc.scalar.activation(rd[:], ln_d[:], AF.Exp, scale=-1.0)
            rdbc = stageps.tile([64, 512], F32, tag="stage")
            nc.tensor.matmul(rdbc[:], c125[:], rd[:], start=True, stop=True)
            ot_sb = otsbpool.tile([64, 512], F32, tag="otsb")
            nc.scalar.copy(ot_sb[:], ot[:])
            on_sb = onsbpool.tile([64, 512], F32, tag="onsb")
            nc.vector.tensor_mul(on_sb[:], ot_sb[:], rdbc[:])
            # transpose back to [q, H] and store
            onat = stageps.tile([128, 256], F32, tag="stage")
            for cc in range(GROUP):
                nc.tensor.transpose(
                    onat[:, 64 * cc:64 * (cc + 1)],
                    on_sb[:, 128 * cc:128 * (cc + 1)], ident_f[:64, :64])
            osb = outpool.tile([128, 256], F32, tag="osb")
            nc.vector.tensor_copy(osb[:], onat[:])
            nc.sync.dma_start(
                out[b].rearrange("(c p) h -> p c h", p=128)
                   [:, GROUP * g:GROUP * (g + 1), :],
                osb[:].rearrange("p (c h) -> p c h", c=GROUP))


_CACHE = {}


def _get_nc():
    if "nc" not in _CACHE:
        nc = bass.Bass("TRN2", target_bir_lowering=False)
        xt = nc.dram_tensor("xt", [BPC, D, T], F32R, kind="ExternalInput")
        wqk = nc.dram_tensor("wqk", [D, 128], F32R, kind="ExternalInput")
        wv = nc.dram_tensor("wv", [D, H], F32R, kind="ExternalInput")
        ut = nc.dram_tensor("ut", [BPC, T, T], F32, kind="ExternalInput")
        out = nc.dram_tensor("out", [BPC, T, H], F32, kind="ExternalOutput")
        with tile.TileContext(nc) as tc:
            with ExitStack() as ctx:
                _build(ctx, tc, xt.ap(), wqk.ap(), wv.ap(), ut.ap(), out.ap())
        _split_excess_waits(nc)
        _CACHE["nc"] = nc
    return _CACHE["nc"]


def kernel(x, Wq, Wk, Wv, drop_u, _trace=False):
    x = np.asarray(x, dtype=np.float32)
    Wq = np.asarray(Wq, dtype=np.float32)
    Wk = np.asarray(Wk, dtype=np.float32)
    Wv = np.asarray(Wv, dtype=np.float32)
    drop_u = np.asarray(drop_u, dtype=np.float32)

    nc = _get_nc()
    xt = np.ascontiguousarray(x.transpose(0, 2, 1))        # [B, D, T]
    ut = np.ascontiguousarray(drop_u.transpose(0, 2, 1))   # [B, T_s, T_q]
    wqk = np.ascontiguousarray(np.concatenate([Wq, Wk], axis=1))  # [D, 128]
    in_maps = []
    for c in range(N_CORES):
        lo = BPC * c
        in_maps.append({
            "xt": xt[lo:lo + BPC],
            "wqk": wqk, "wv": Wv,
            "ut": ut[lo:lo + BPC],
        })
    res = run_bass_kernel_spmd(
        nc, in_maps, core_ids=list(range(N_CORES)), trace=_trace)
    out = np.concatenate([res.results[c]["out"] for c in range(N_CORES)], axis=0)
    if _trace:
        kernel.last_exec_time_ns = res.exec_time_ns
        kernel.last_results = res
    return out
